# revision 6
# baseline (speedup 1.0000x reference)
"""ChebyKANLinear Trainium2 kernel.

Math: y[b,o] = (1/I) * sum_{i,d} T_d(c[b,i]) * W[i,o,d],  c = clip(tanh(x), -1+eps, 1-eps)
with Chebyshev T_0=1, T_1=c, T_2=2c^2-1, T_3=4c^3-3c.

Re-expressed in the monomial basis (exact linear recombination, folded into the
weights on the host):
    y = bias + c @ V1 + c^2 @ V2 + c^3 @ V3
    V1 = (W1 - 3*W3)/I, V2 = 2*W2/I, V3 = 4*W3/I, bias_o = sum_i (W0 - W2)[i,o]/I

Sharding: data-parallel over the batch dim across 8 NeuronCores (x transposed on
host so the contraction dim i lands on SBUF partitions); V/bias replicated.
On-device per core: tanh (ACT), clip/square/cube (DVE), then 2x7 accumulating
matmuls into PSUM (a K=1 ones-matmul broadcasts the bias), scale-free copy out.
"""

from contextlib import ExitStack

import numpy as np

import concourse.bass as bass
import concourse.tile as tile
from concourse import bacc, mybir
from concourse.bass_utils import run_bass_kernel_spmd

N_CORES = 8
B, I, O, D = 2048, 256, 256, 4
BL = B // N_CORES  # batch rows per core
EPS = 1e-07
F32 = mybir.dt.float32

_cache = {}


def _build_program():
    nc = bacc.Bacc("TRN2", target_bir_lowering=False, debug=False, num_devices=N_CORES)

    # [i_half, i_in_half, b_local]  (x slice pre-transposed on host)
    xt_d = nc.dram_tensor("xt", [2, 128, BL], F32, kind="ExternalInput")
    # [d-1, i_half, i_in_half, o]
    v_d = nc.dram_tensor("v", [3, 2, 128, O], F32, kind="ExternalInput")
    bias_d = nc.dram_tensor("bias", [1, O], F32, kind="ExternalInput")
    # [b_half, b_in_half, o]
    y_d = nc.dram_tensor("y", [BL // 128, 128, O], F32, kind="ExternalOutput")

    # python floats holding the exact f32-rounded clip bounds
    clip_lo = float(np.float32(-1.0 + EPS))
    clip_hi = float(np.float32(1.0 - EPS))

    with tile.TileContext(nc) as tc, ExitStack() as ctx:
        pool = ctx.enter_context(tc.tile_pool(name="main", bufs=1))
        psum = ctx.enter_context(
            tc.tile_pool(name="psum", bufs=1, space=bass.MemorySpace.PSUM)
        )

        # constants for the bias broadcast matmul and PE warmup
        ones = pool.tile([1, 128], F32, tag="ones")
        nc.gpsimd.memset(ones[:], 1.0)
        ones_row = pool.tile([1, 512], F32, tag="ones_row")
        nc.gpsimd.memset(ones_row[:], 1.0)

        xt = {}
        for ih in range(2):
            xt[ih] = pool.tile([128, BL], F32, tag=f"xt{ih}", name=f"xt{ih}")
        vt = {}
        for d in range(3):
            for ih in range(2):
                vt[(d, ih)] = pool.tile(
                    [128, O], F32, tag=f"v{d}{ih}", name=f"v{d}{ih}"
                )
        bias_t = pool.tile([1, O], F32, tag="bias")

        # Two HWDGE queues: sync(SP) carries x + late V tiles; scalar(ACT)
        # carries bias + early V tiles, then frees up for the tanhs.
        # gpsimd SWDGE is slow (~ms-scale sem latency) -> data never rides it.
        nc.sync.dma_start(xt[0][:], xt_d[0])
        nc.sync.dma_start(xt[1][:], xt_d[1])
        nc.scalar.dma_start(bias_t[:], bias_d[:])
        nc.scalar.dma_start(vt[(0, 0)][:], v_d[0, 0])
        nc.scalar.dma_start(vt[(0, 1)][:], v_d[0, 1])
        nc.sync.dma_start(vt[(1, 0)][:], v_d[1, 0])
        nc.sync.dma_start(vt[(1, 1)][:], v_d[1, 1])
        nc.sync.dma_start(vt[(2, 0)][:], v_d[2, 0])

        # PE warmup: dummy K=1 matmuls into a scratch PSUM bank while DMAs
        # stream, so the HAM clock-gate opens (1.2 -> 2.4 GHz) before the
        # real accumulation chain runs.
        scratch = psum.tile([128, 512], F32, tag="scratch")
        for w in range(6):
            nc.tensor.matmul(
                scratch[:], ones[:], ones_row[:], start=True, stop=True
            )

        n_bh = BL // 128
        acc = {}
        for bh in range(n_bh):
            acc[bh] = psum.tile([128, O], F32, tag=f"acc{bh}", name=f"acc{bh}")
            # bias broadcast: [128,1] ones.T @ [1,O] bias
            nc.tensor.matmul(acc[bh][:], ones[:], bias_t[:], start=True, stop=False)

        # basis: c = tanh(x) (no clip needed: |tanh|max ~= 0.99992 < clip
        # bound for this input distribution, and the monomial form is smooth
        # through +-1 so the bound is numerically irrelevant), c2/c3 on DVE
        basis = {}
        for ih in range(2):
            c = pool.tile([128, BL], F32, tag=f"c{ih}")
            nc.scalar.activation(c[:], xt[ih][:], mybir.ActivationFunctionType.Tanh)
            basis[(0, ih)] = c
        for ih in range(2):
            c2 = pool.tile([128, BL], F32, tag=f"c2{ih}")
            nc.vector.tensor_mul(c2[:], basis[(0, ih)][:], basis[(0, ih)][:])
            basis[(1, ih)] = c2
        for ih in range(2):
            c3 = pool.tile([128, BL], F32, tag=f"c3{ih}")
            nc.vector.tensor_mul(c3[:], basis[(1, ih)][:], basis[(0, ih)][:])
            basis[(2, ih)] = c3

        # late V tiles ride the freed-up scalar queue
        nc.scalar.dma_start(vt[(2, 1)][:], v_d[2, 1])

        # accumulation chain, ordered by operand arrival
        mm_order = [(0, 0), (1, 0), (0, 1), (1, 1), (2, 0), (2, 1)]
        for d, ih in mm_order:
            for bh in range(n_bh):
                nc.tensor.matmul(
                    acc[bh][:],
                    basis[(d, ih)][:, bh * 128 : (bh + 1) * 128],
                    vt[(d, ih)][:],
                    start=False,
                    stop=(d == 2 and ih == 1),
                )
        for bh in range(n_bh):
            y_sb = pool.tile([128, O], F32, tag=f"y{bh}")
            nc.vector.tensor_copy(y_sb[:], acc[bh][:])
            nc.sync.dma_start(y_d[bh], y_sb[:])

    nc.compile()
    return nc


def _get_program():
    if "nc" not in _cache:
        _cache["nc"] = _build_program()
    return _cache["nc"]


def kernel(x, cheby_coeffs):
    x = np.ascontiguousarray(x, dtype=np.float32)
    W = np.ascontiguousarray(cheby_coeffs, dtype=np.float32)
    assert x.shape == (B, I) and W.shape == (I, O, D)

    inv_i = np.float32(1.0 / I)
    V = np.stack(
        [
            W[:, :, 1] - 3.0 * W[:, :, 3],
            2.0 * W[:, :, 2],
            4.0 * W[:, :, 3],
        ]
    ).astype(np.float32) * inv_i  # [3, I, O]
    v_arr = np.ascontiguousarray(V.reshape(3, 2, 128, O))
    bias = ((W[:, :, 0] - W[:, :, 2]).sum(axis=0, dtype=np.float32) * inv_i).reshape(
        1, O
    )
    bias = np.ascontiguousarray(bias, dtype=np.float32)

    nc = _get_program()
    in_maps = []
    for c_id in range(N_CORES):
        xs = x[c_id * BL : (c_id + 1) * BL, :]  # [BL, I]
        xt = np.ascontiguousarray(xs.T).reshape(2, 128, BL)
        in_maps.append({"xt": xt, "v": v_arr, "bias": bias})

    res = run_bass_kernel_spmd(nc, in_maps, list(range(N_CORES)))
    y = np.concatenate(
        [r["y"].reshape(BL, O) for r in res.results], axis=0
    )
    return y


# revision 8
# speedup vs baseline: 1.3337x; 1.3337x over previous
"""ChebyKANLinear Trainium2 kernel.

Math: y[b,o] = (1/I) * sum_{i,d} T_d(c[b,i]) * W[i,o,d],  c = tanh(x)
with Chebyshev T_0=1, T_1=c, T_2=2c^2-1, T_3=4c^3-3c.
(The reference also clips c to [-1+1e-7, 1-1e-7] before arccos; in the
monomial form below the bound is numerically irrelevant — |tanh|max for this
input distribution is 0.99992, far below it — so the clip is dropped.)

Re-expressed in the monomial basis (exact linear recombination, folded into
the weights on the host):
    y = bias + c @ V1 + c^2 @ V2 + c^3 @ V3
    V1 = (W1 - 3*W3)/I, V2 = 2*W2/I, V3 = 4*W3/I, bias_o = sum_i (W0 - W2)[i,o]/I

Sharding: 2D — batch into 4 shards x output_dim into 2 shards across the 8
NeuronCores. Per core the matmuls are computed TRANSPOSED,
    yT[o, b] = sum_k  V_k[i, o].T @ (c^k)[i, b]
so each core runs only 6 fp32 matmuls of [K=128, M=128, N=512] (N=512 is the
fp32 moving-operand max — fewest passes through the PE for this contraction),
and the bias becomes a per-partition scalar fused into the PSUM->SBUF copy
(vector.tensor_scalar_add) instead of costing extra matmuls.

x arrives pre-transposed from the host (contraction dim i on SBUF
partitions); the host transposes each core's yT back and reassembles.
DMAs ride the two fast HWDGE queues (SP + ACT), ordered so the ACT engine is
free exactly when the tanh activation table finishes loading.
"""

from contextlib import ExitStack

import numpy as np

import concourse.bass as bass
import concourse.tile as tile
from concourse import bacc, mybir
from concourse.bass_utils import run_bass_kernel_spmd

N_CORES = 8
B, I, O, D = 2048, 256, 256, 4
RB, SO = 4, 2  # batch shards x output shards
BL = B // RB  # 512 batch rows per core
OL = O // SO  # 128 output cols per core
F32 = mybir.dt.float32

_cache = {}


def _build_program():
    nc = bacc.Bacc("TRN2", target_bir_lowering=False, debug=False, num_devices=N_CORES)

    # [i_half, i_in_half, b_local]  (x slice pre-transposed on host)
    xt_d = nc.dram_tensor("xt", [2, 128, BL], F32, kind="ExternalInput")
    # [d-1, i_half, i_in_half, o_local]
    v_d = nc.dram_tensor("v", [3, 2, 128, OL], F32, kind="ExternalInput")
    # bias for this core's o-slice, as a per-partition column
    bias_d = nc.dram_tensor("bias", [OL, 1], F32, kind="ExternalInput")
    # transposed output [o_local, b_local]
    y_d = nc.dram_tensor("y", [OL, BL], F32, kind="ExternalOutput")

    with tile.TileContext(nc) as tc, ExitStack() as ctx:
        pool = ctx.enter_context(tc.tile_pool(name="main", bufs=1))
        psum = ctx.enter_context(
            tc.tile_pool(name="psum", bufs=1, space=bass.MemorySpace.PSUM)
        )

        xt = {}
        for ih in range(2):
            xt[ih] = pool.tile([128, BL], F32, tag=f"xt{ih}", name=f"xt{ih}")
        vt = {}
        for d in range(3):
            for ih in range(2):
                vt[(d, ih)] = pool.tile(
                    [128, OL], F32, tag=f"v{d}{ih}", name=f"v{d}{ih}"
                )
        bias_t = pool.tile([OL, 1], F32, tag="bias")

        # sync(SP) queue: the big x tiles + late V tiles.
        # scalar(ACT) queue: small early V tiles + bias, then free for tanh
        # right when the activation table finishes loading (~1.3us).
        nc.sync.dma_start(xt[0][:], xt_d[0])
        nc.sync.dma_start(xt[1][:], xt_d[1])
        nc.scalar.dma_start(vt[(0, 0)][:], v_d[0, 0])
        nc.scalar.dma_start(vt[(0, 1)][:], v_d[0, 1])
        nc.scalar.dma_start(bias_t[:], bias_d[:])
        nc.sync.dma_start(vt[(1, 0)][:], v_d[1, 0])
        nc.sync.dma_start(vt[(1, 1)][:], v_d[1, 1])
        nc.sync.dma_start(vt[(2, 0)][:], v_d[2, 0])
        nc.sync.dma_start(vt[(2, 1)][:], v_d[2, 1])

        # basis: c = tanh(xT) on ACT, c^2/c^3 on DVE
        basis = {}
        for ih in range(2):
            c = pool.tile([128, BL], F32, tag=f"c{ih}")
            nc.scalar.activation(c[:], xt[ih][:], mybir.ActivationFunctionType.Tanh)
            basis[(0, ih)] = c
        for ih in range(2):
            c2 = pool.tile([128, BL], F32, tag=f"c2{ih}")
            nc.vector.tensor_mul(c2[:], basis[(0, ih)][:], basis[(0, ih)][:])
            basis[(1, ih)] = c2
        for ih in range(2):
            c3 = pool.tile([128, BL], F32, tag=f"c3{ih}")
            nc.vector.tensor_mul(c3[:], basis[(1, ih)][:], basis[(0, ih)][:])
            basis[(2, ih)] = c3

        # yT[o, b] accumulation: 6 matmuls, ordered by operand arrival
        acc = psum.tile([128, BL], F32, tag="acc")
        mm_order = [(0, 0), (0, 1), (1, 0), (1, 1), (2, 0), (2, 1)]
        for n, (d, ih) in enumerate(mm_order):
            nc.tensor.matmul(
                acc[:OL, :],
                vt[(d, ih)][:],
                basis[(d, ih)][:],
                start=(n == 0),
                stop=(n == len(mm_order) - 1),
            )

        # PSUM -> SBUF with the bias (per-partition scalar) fused in
        y_sb = pool.tile([OL, BL], F32, tag="y_sb")
        nc.vector.tensor_scalar_add(y_sb[:], acc[:OL, :], bias_t[:])
        nc.sync.dma_start(y_d[:], y_sb[:])

    nc.compile()
    return nc


def _get_program():
    if "nc" not in _cache:
        _cache["nc"] = _build_program()
    return _cache["nc"]


def _make_in_maps(x, cheby_coeffs):
    x = np.ascontiguousarray(x, dtype=np.float32)
    W = np.ascontiguousarray(cheby_coeffs, dtype=np.float32)
    assert x.shape == (B, I) and W.shape == (I, O, D)

    inv_i = np.float32(1.0 / I)
    V = np.stack(
        [
            W[:, :, 1] - 3.0 * W[:, :, 3],
            2.0 * W[:, :, 2],
            4.0 * W[:, :, 3],
        ]
    ).astype(np.float32) * inv_i  # [3, I, O]
    bias_full = (W[:, :, 0] - W[:, :, 2]).sum(axis=0, dtype=np.float32) * inv_i  # [O]

    in_maps = []
    xt_shards = []
    for rb in range(RB):
        xs = x[rb * BL : (rb + 1) * BL, :]  # [BL, I]
        xt_shards.append(np.ascontiguousarray(xs.T).reshape(2, 128, BL))
    v_shards = []
    bias_shards = []
    for so in range(SO):
        v_shards.append(
            np.ascontiguousarray(V[:, :, so * OL : (so + 1) * OL]).reshape(
                3, 2, 128, OL
            )
        )
        bias_shards.append(
            np.ascontiguousarray(bias_full[so * OL : (so + 1) * OL].reshape(OL, 1))
        )
    for c_id in range(N_CORES):
        rb, so = divmod(c_id, SO)
        in_maps.append(
            {"xt": xt_shards[rb], "v": v_shards[so], "bias": bias_shards[so]}
        )
    return in_maps


def kernel(x, cheby_coeffs):
    nc = _get_program()
    in_maps = _make_in_maps(x, cheby_coeffs)
    res = run_bass_kernel_spmd(nc, in_maps, list(range(N_CORES)))
    y = np.empty((B, O), dtype=np.float32)
    for c_id in range(N_CORES):
        rb, so = divmod(c_id, SO)
        y[rb * BL : (rb + 1) * BL, so * OL : (so + 1) * OL] = res.results[c_id]["y"].T
    return y


# revision 9
# speedup vs baseline: 1.4146x; 1.0607x over previous
"""ChebyKANLinear Trainium2 kernel.

Math: y[b,o] = (1/I) * sum_{i,d} T_d(c[b,i]) * W[i,o,d],  c = tanh(x)
with Chebyshev T_0=1, T_1=c, T_2=2c^2-1, T_3=4c^3-3c.
(The reference also clips c to [-1+1e-7, 1-1e-7] before arccos; in the
monomial form below the bound is numerically irrelevant — |tanh|max for this
input distribution is 0.99992, far below it — so the clip is dropped.)

Re-expressed in the monomial basis (exact linear recombination, folded into
the weights on the host):
    y = bias + c @ V1 + c^2 @ V2 + c^3 @ V3
    V1 = (W1 - 3*W3)/I, V2 = 2*W2/I, V3 = 4*W3/I, bias_o = sum_i (W0 - W2)[i,o]/I

Sharding: 2D — batch into 4 shards x output_dim into 2 shards across the 8
NeuronCores. Per core the matmuls are computed TRANSPOSED,
    yT[o, b] = sum_k  V_k[i, o].T @ (c^k)[i, b]
so each core runs only 6 fp32 matmuls of [K=128, M=128, N=512] (N=512 is the
fp32 moving-operand max — fewest PE passes for this contraction), and the
bias becomes a per-partition scalar fused into the PSUM->SBUF copy
(vector.tensor_scalar_add) instead of costing extra matmuls.

Perf notes baked in from trace analysis:
- All of V plus the bias column ride ONE wide-row dma_start ([128, 769] ->
  3KB/partition rows); narrow-row DMAs measured ~3x slower per byte.
- x rides two dma_starts on the other HWDGE queue (sync/SP).
- Two real-shaped (K=128, N=512) warmup matmuls on memset tiles run during
  the DMA phase so the PE HAM clock-gate (1.2 -> 2.4 GHz) opens right as the
  real accumulation chain peaks.
- Output is written as two half DMAs on the two queues to overlap the
  PSUM->SBUF bias-add with the store.
"""

from contextlib import ExitStack

import numpy as np

import concourse.bass as bass
import concourse.tile as tile
from concourse import bacc, mybir
from concourse.bass_utils import run_bass_kernel_spmd

N_CORES = 8
B, I, O, D = 2048, 256, 256, 4
RB, SO = 4, 2  # batch shards x output shards
BL = B // RB  # 512 batch rows per core
OL = O // SO  # 128 output cols per core
F32 = mybir.dt.float32

_cache = {}


def _build_program():
    nc = bacc.Bacc("TRN2", target_bir_lowering=False, debug=False, num_devices=N_CORES)

    # [i_half, i_in_half, b_local]  (x slice pre-transposed on host)
    xt_d = nc.dram_tensor("xt", [2, 128, BL], F32, kind="ExternalInput")
    # packed weights: col (ih*3+d)*OL + o holds V[d, ih*128+i, o]; col 768 = bias
    vb_d = nc.dram_tensor("vb", [128, 6 * OL + 1], F32, kind="ExternalInput")
    # transposed output [o_local, b_local]
    y_d = nc.dram_tensor("y", [OL, BL], F32, kind="ExternalOutput")

    with tile.TileContext(nc) as tc, ExitStack() as ctx:
        pool = ctx.enter_context(tc.tile_pool(name="main", bufs=1))
        psum = ctx.enter_context(
            tc.tile_pool(name="psum", bufs=1, space=bass.MemorySpace.PSUM)
        )

        # PE warmup operands (DVE is idle this early; values are irrelevant)
        wu_w = pool.tile([128, 128], F32, tag="wu_w")
        nc.vector.memset(wu_w[:], 1.0)
        wu_r = pool.tile([128, 512], F32, tag="wu_r")
        nc.vector.memset(wu_r[:], 1.0)

        vb = pool.tile([128, 6 * OL + 1], F32, tag="vb")
        nc.scalar.dma_start(vb[:], vb_d[:])
        xt = {}
        for ih in range(2):
            xt[ih] = pool.tile([128, BL], F32, tag=f"xt{ih}", name=f"xt{ih}")
        nc.sync.dma_start(xt[0][:], xt_d[0])
        nc.sync.dma_start(xt[1][:], xt_d[1])

        # warmup matmuls: dense K=128 N=512 so HAM sees real PE activity
        wu_acc = psum.tile([128, 512], F32, tag="wu_acc")
        for _ in range(2):
            nc.tensor.matmul(wu_acc[:], wu_w[:], wu_r[:], start=True, stop=True)

        # basis: c = tanh(xT) on ACT, c^2/c^3 on DVE
        basis = {}
        for ih in range(2):
            c = pool.tile([128, BL], F32, tag=f"c{ih}")
            nc.scalar.activation(c[:], xt[ih][:], mybir.ActivationFunctionType.Tanh)
            basis[(0, ih)] = c
        for ih in range(2):
            c2 = pool.tile([128, BL], F32, tag=f"c2{ih}")
            nc.vector.tensor_mul(c2[:], basis[(0, ih)][:], basis[(0, ih)][:])
            basis[(1, ih)] = c2
        for ih in range(2):
            c3 = pool.tile([128, BL], F32, tag=f"c3{ih}")
            nc.vector.tensor_mul(c3[:], basis[(1, ih)][:], basis[(0, ih)][:])
            basis[(2, ih)] = c3

        # yT[o, b] accumulation: 6 matmuls, ordered by operand arrival
        acc = psum.tile([128, BL], F32, tag="acc")
        mm_order = [(0, 0), (0, 1), (1, 0), (1, 1), (2, 0), (2, 1)]
        for n, (d, ih) in enumerate(mm_order):
            col = (ih * 3 + d) * OL
            nc.tensor.matmul(
                acc[:OL, :],
                vb[:, col : col + OL],
                basis[(d, ih)][:],
                start=(n == 0),
                stop=(n == len(mm_order) - 1),
            )

        # PSUM -> SBUF with bias fused (per-partition scalar), halves
        # pipelined into two output DMAs on the two queues
        bias_col = vb[:, 6 * OL : 6 * OL + 1]
        y_sb = pool.tile([OL, BL], F32, tag="y_sb")
        half = BL // 2
        nc.vector.tensor_scalar_add(
            y_sb[:, :half], acc[:OL, :half], bias_col
        )
        nc.sync.dma_start(y_d[:, :half], y_sb[:, :half])
        nc.vector.tensor_scalar_add(
            y_sb[:, half:], acc[:OL, half:], bias_col
        )
        nc.scalar.dma_start(y_d[:, half:], y_sb[:, half:])

    nc.compile()
    return nc


def _get_program():
    if "nc" not in _cache:
        _cache["nc"] = _build_program()
    return _cache["nc"]


def _make_in_maps(x, cheby_coeffs):
    x = np.ascontiguousarray(x, dtype=np.float32)
    W = np.ascontiguousarray(cheby_coeffs, dtype=np.float32)
    assert x.shape == (B, I) and W.shape == (I, O, D)

    inv_i = np.float32(1.0 / I)
    V = np.stack(
        [
            W[:, :, 1] - 3.0 * W[:, :, 3],
            2.0 * W[:, :, 2],
            4.0 * W[:, :, 3],
        ]
    ).astype(np.float32) * inv_i  # [3, I, O]
    bias_full = (W[:, :, 0] - W[:, :, 2]).sum(axis=0, dtype=np.float32) * inv_i  # [O]

    xt_shards = []
    for rb in range(RB):
        xs = x[rb * BL : (rb + 1) * BL, :]  # [BL, I]
        xt_shards.append(np.ascontiguousarray(xs.T).reshape(2, 128, BL))
    vb_shards = []
    for so in range(SO):
        vb = np.empty((128, 6 * OL + 1), dtype=np.float32)
        for ih in range(2):
            for d in range(3):
                col = (ih * 3 + d) * OL
                # vb[i, col+o] = V[d, ih*128+i, so*OL+o]
                vb[:, col : col + OL] = V[
                    d, ih * 128 : (ih + 1) * 128, so * OL : (so + 1) * OL
                ]
        vb[:, 6 * OL] = bias_full[so * OL : (so + 1) * OL]
        vb_shards.append(vb)
    in_maps = []
    for c_id in range(N_CORES):
        rb, so = divmod(c_id, SO)
        in_maps.append({"xt": xt_shards[rb], "vb": vb_shards[so]})
    return in_maps


def kernel(x, cheby_coeffs):
    nc = _get_program()
    in_maps = _make_in_maps(x, cheby_coeffs)
    res = run_bass_kernel_spmd(nc, in_maps, list(range(N_CORES)))
    y = np.empty((B, O), dtype=np.float32)
    for c_id in range(N_CORES):
        rb, so = divmod(c_id, SO)
        y[rb * BL : (rb + 1) * BL, so * OL : (so + 1) * OL] = res.results[c_id]["y"].T
    return y
